# revision 16
# baseline (speedup 1.0000x reference)
"""Multi-head self-attention on 8 Trainium2 NeuronCores.

Problem: B=2, L=2048, E=1024, H=16 heads, D=64 (fp32 in/out).
Sharding: 2-way batch x 4-way head-group. Core c handles batch c//4 and
heads 4*(c%4) .. 4*(c%4)+3 (a 256-wide slice of the QKV output dim).
Each core computes a partial output y_c = Attn_c @ W_O[slice]; the host
sums the 4 partials per batch (the "all-reduce" of row-parallel W_O).

Perf design (v6):
 - All matmul operands bf16 (1 cycle/row, FWL weight loads, half the
   HBM traffic). Output DMA'd as bf16 too; host accumulates in fp32.
 - Host pre-packs every tensor partition-major so each DMA descriptor
   is a 2-4KB contiguous run; inputs stream as 24 x 256KB chunk
   transfers paced by the x-pool's WAR rotation, so completion order
   matches need order (the DMA engines fair-queue concurrent
   transfers — unpaced, the first-needed chunk finishes last).
 - The serial ScalarE exp stream (~1.4us x 96 calls) is the long pole;
   everything else is arranged to start it as early as possible and
   keep it dense:
     * K/Q projections are split by l-half: after only half the K/Q
       input (4.2MB) has landed, head 0's first scores/exp stages for
       q-tiles 0-1 / k-chunks 0-7 fire while the second halves project
       underneath (2-PSUM-bank projection groups leave the scores'
       6 banks free).
     * V projection + PE transposes ride under later early stages.
     * PV lags exp by N_EARLY stages (et pool sized to match) and
       catches up at the tail.
 - Scores are computed transposed, St = [k, q]; softmax denominator
   comes free as row 64 of the PV psum via a ones column in V (V tiles
   padded to 128 cols so PV LDWEIGHTS can use fast-weight-load).
 - exp on ScalarE with the 1/sqrt(D) scale folded in; no max
   subtraction (logits bounded ~|4|, exp can't overflow). Input DMAs
   issue only on sync/gpsimd queues: a dma_start on the scalar queue
   would stall the exp stream behind descriptor generation.
 - B_V is folded on the host: softmax rows sum to 1, so the V bias adds
   the constant row B_V @ W_O to the output.
"""

import sys

if "/opt/trn_rl_repo" not in sys.path:
    sys.path.insert(0, "/opt/trn_rl_repo")

import numpy as np
import ml_dtypes

B, L, E = 2, 2048, 1024
H, D = 16, 64
OC = 256          # per-core slice of the H*D output dim (4 heads)
HC = OC // D      # heads per core = 4
ECH = E // 128    # 8 e-chunks
LT = L // 512     # 4 l-tiles of 512
KC = L // 128     # 16 k-chunks
GRP = [3, 3, 2, 3, 3, 2]   # k-chunk grouping per ScalarE exp call
N_EARLY = 14      # PV lag in stages (covers lh1-proj + V + transposes)
N_WARM = 16       # warm-up matmuls at t=0 (HAM release + DMA-ramp cover)

_CACHE = {}


def _build():
    from concourse import bacc, tile, mybir
    from concourse import masks

    f32 = mybir.dt.float32
    bf16 = mybir.dt.bfloat16
    Exp = mybir.ActivationFunctionType.Exp

    nc = bacc.Bacc("TRN2", target_bir_lowering=False, debug=False)

    # partition-major packed inputs: [p, half, c, l] with e-chunk = c*2+half
    qT = nc.dram_tensor("qT", [128, 2, 4, L], bf16, kind="ExternalInput").ap()
    kT = nc.dram_tensor("kT", [128, 2, 4, L], bf16, kind="ExternalInput").ap()
    vT = nc.dram_tensor("vT", [128, 2, 4, L], bf16, kind="ExternalInput").ap()
    wq = nc.dram_tensor("wq", [128, ECH, OC], bf16, kind="ExternalInput").ap()
    wk = nc.dram_tensor("wk", [128, ECH, OC], bf16, kind="ExternalInput").ap()
    wv = nc.dram_tensor("wv", [128, ECH, OC], bf16, kind="ExternalInput").ap()
    wo = nc.dram_tensor("wo", [128, 2, E], bf16, kind="ExternalInput").ap()
    bq = nc.dram_tensor("bq", [128, 2, 1], f32, kind="ExternalInput").ap()
    bk = nc.dram_tensor("bk", [128, 2, 1], f32, kind="ExternalInput").ap()
    yT = nc.dram_tensor("yT", [E, L], bf16, kind="ExternalOutput").ap()

    with tile.TileContext(nc) as tc:
        with (
            tc.tile_pool(name="w", bufs=1) as wp,
            tc.tile_pool(name="xt", bufs=20) as xp,
            tc.tile_pool(name="qk", bufs=1) as qkp,
            tc.tile_pool(name="vt", bufs=1) as vtp,
            tc.tile_pool(name="et", bufs=N_EARLY + 3) as ep,
            tc.tile_pool(name="norm", bufs=2) as npl,
            tc.tile_pool(name="yst", bufs=4) as ysp,
        ):
            # ---- PE warm-up: matmuls on (mostly) uninitialized SBUF —
            # the result is never read; a 1-column memset allocates the
            # tile without gating the matmuls on a full-tile write ----
            warm = wp.tile([128, 512], bf16, tag="warm")
            nc.vector.memset(warm[:, 0:1], 0.0)
            with tc.tile_pool(name="ps_wu", bufs=1, space="PSUM") as pwu:
                pw = pwu.tile([128, 512], f32, tag="pw")
                for i in range(N_WARM):
                    nc.tensor.matmul(pw[:], warm[:, 0:128], warm[:],
                                     start=True, stop=True)

            # ---- weights + biases resident (K first: it's needed first) ----
            twk = wp.tile([128, ECH, OC], bf16, tag="twk")
            twq = wp.tile([128, ECH, OC], bf16, tag="twq")
            twv = wp.tile([128, ECH, OC], bf16, tag="twv")
            two = wp.tile([128, 2, E], bf16, tag="two")
            tbq = wp.tile([128, 2, 1], f32, tag="tbq")
            tbk = wp.tile([128, 2, 1], f32, tag="tbk")
            nc.sync.dma_start(twk[:], wk)
            nc.gpsimd.dma_start(tbk[:], bk)

            # ---- persistent activations ----
            # K/Q kept as separate tiles per (m, l-half) so the early
            # attention stages only depend on the lh0 projections.
            qt_t = [[qkp.tile([128, 1024], bf16, tag=f"qt{m}_{lh}",
                              name=f"qt{m}_{lh}") for lh in range(2)]
                    for m in range(2)]
            kt_t = [[qkp.tile([128, 1024], bf16, tag=f"kt{m}_{lh}",
                              name=f"kt{m}_{lh}") for lh in range(2)]
                    for m in range(2)]
            ot_t = [qkp.tile([128, L], bf16, tag=f"ot{m}", name=f"ot{m}")
                    for m in range(2)]
            # V with a ones column per head: one tile per l-tile of 4
            # k-chunks, [l, kc4, h, 128] (padded to 128 cols for FWL)
            v_t4 = [vtp.tile([128, 4, HC, 128], bf16, tag=f"v{i}", name=f"v{i}")
                    for i in range(LT)]
            for i in range(LT):
                nc.vector.memset(v_t4[i][:, :, :, D:], 0.0)
                nc.vector.memset(v_t4[i][:, :, :, D:D + 1], 1.0)

            in_engs = [nc.sync, nc.gpsimd]
            out_engs = [nc.sync, nc.gpsimd, nc.scalar]
            rr = [0, 0]

            def dma_in(dst, src):
                in_engs[rr[0] % 2].dma_start(dst, src)
                rr[0] += 1

            def dma_out(dst, src):
                out_engs[rr[1] % 3].dma_start(dst, src)
                rr[1] += 1

            # ---- x chunk streaming: [128, 1024] tiles, one per
            # (tensor, half, c, lh), DMA'd in need order ----
            xch = {}

            def x_chunks(name, src, lh):
                for half in range(2):
                    for c in range(4):
                        x = xp.tile([128, 1024], bf16, tag="x",
                                    name=f"x{name}_{half}_{c}_{lh}")
                        dma_in(x[:], src[:, half, c, lh * 1024:(lh + 1) * 1024])
                        xch[(name, half, c, lh)] = x

            x_chunks("k", kT, 0)
            nc.sync.dma_start(twq[:], wq)
            nc.gpsimd.dma_start(tbq[:], bq)
            x_chunks("q", qT, 0)
            x_chunks("k", kT, 1)
            x_chunks("q", qT, 1)

            # ---- lh0 projections, chunk-major: every arriving x chunk
            # immediately feeds one matmul of each of the 4 (m, lt<2)
            # accumulation groups, so the PE tracks the DMA stream instead
            # of waiting for a full tensor. Needs its own 4-bank pool,
            # closed before ps_st opens (LIFO). ----
            with tc.tile_pool(name="ps_kq0", bufs=4, space="PSUM") as pskq0:
                for name, wt, tb, dst in (("k", twk, tbk, kt_t),
                                          ("q", twq, tbq, qt_t)):
                    pps = {(m, lt): pskq0.tile([128, 512], f32, tag="pp0",
                                               name=f"pp0{name}_{m}_{lt}")
                           for m in range(2) for lt in range(2)}
                    for half in range(2):
                        for c in range(4):
                            e = c * 2 + half
                            for m in range(2):
                                for lt in range(2):
                                    nc.tensor.matmul(
                                        pps[(m, lt)][:],
                                        wt[:, e, m * 128:(m + 1) * 128],
                                        xch[(name, half, c, 0)][:, lt * 512:(lt + 1) * 512],
                                        start=(half == 0 and c == 0),
                                        stop=(half == 1 and c == 3))
                    for m in range(2):
                        for lt in range(2):
                            nc.vector.tensor_scalar_add(
                                dst[m][0][:, lt * 512:(lt + 1) * 512],
                                pps[(m, lt)][:], tb[:, m, :])

            pst_cm = tc.tile_pool(name="ps_st", bufs=2, space="PSUM")
            pst = pst_cm.__enter__()
            pskq_cm = tc.tile_pool(name="ps_kq", bufs=2, space="PSUM")
            pskq = pskq_cm.__enter__()

            def proj_group(name, wt, tb, dst, m, lt):
                pp = pskq.tile([128, 512], f32, tag="pp",
                               name=f"pp{name}_{m}_{lt}")
                lh, sl = lt // 2, (lt % 2)
                for e in range(ECH):
                    nc.tensor.matmul(
                        pp[:], wt[:, e, m * 128:(m + 1) * 128],
                        xch[(name, e % 2, e // 2, lh)][:, sl * 512:(sl + 1) * 512],
                        start=(e == 0), stop=(e == ECH - 1))
                nc.vector.tensor_scalar_add(
                    dst[m][lh][:, sl * 512:(sl + 1) * 512], pp[:], tb[:, m, :])

            # ================= attention stage machinery ==================
            GSEQ = []
            kc0 = 0
            for gi, g in enumerate(GRP):
                GSEQ.append((kc0, g, gi == len(GRP) - 1))
                kc0 += g

            # stage order: head 0 opens with q-tiles 0/1 interleaved and
            # k-half 0 groups first (only lh0 K/Q needed); the rest is the
            # plain (h, qt, gi) sweep.
            stages = []
            for gi in (0, 1, 2):
                for qt in (0, 1):
                    stages.append((0, qt, gi))
            for gi in (3, 4, 5):
                for qt in (0, 1):
                    stages.append((0, qt, gi))
            for qt in (2, 3):
                for gi in range(6):
                    stages.append((0, qt, gi))
            for h in range(1, HC):
                for qt in range(LT):
                    for gi in range(6):
                        stages.append((h, qt, gi))
            NS = len(stages)

            st_t = [None] * NS
            et_t = [None] * NS

            def emit_scores(s):
                h, qt, gi = stages[s]
                kc0, g, _last = GSEQ[gi]
                m, po = h // 2, (h % 2) * 64
                st = pst.tile([128, 3, 512], f32, tag="st", name=f"st{s}")
                st_t[s] = st
                for j in range(g):
                    kc = kc0 + j
                    nc.tensor.matmul(
                        st[:, j, :],
                        kt_t[m][kc // 8][po:po + 64,
                                         (kc % 8) * 128:(kc % 8 + 1) * 128],
                        qt_t[m][qt // 2][po:po + 64,
                                         (qt % 2) * 512:(qt % 2 + 1) * 512],
                        start=True, stop=True)

            def emit_exp(s):
                h, qt, gi = stages[s]
                kc0, g, _last = GSEQ[gi]
                st = st_t[s]
                et = ep.tile([128, 3, 512], bf16, tag="et", name=f"et{s}")
                et_t[s] = et
                nc.scalar.activation(et[:, 0:g, :], st[:, 0:g, :], Exp,
                                     scale=0.125)

            po_t = {}

            def emit_pv(s, pso):
                h, qt, gi = stages[s]
                kc0, g, last = GSEQ[gi]
                m, po = h // 2, (h % 2) * 64
                qs = slice(qt * 512, (qt + 1) * 512)
                et = et_t[s]
                if (h, qt) not in po_t:
                    po_t[(h, qt)] = pso.tile([128, 512], f32, tag="po",
                                             name=f"po{h}_{qt}")
                p_o = po_t[(h, qt)]
                for j in range(g):
                    kc = kc0 + j
                    nc.tensor.matmul(
                        p_o[:], v_t4[kc // 4][:, kc % 4, h, :], et[:, j, :],
                        start=(kc == 0), stop=(kc == KC - 1))
                if last:
                    # normalize: row 64 of p_o holds the denominators
                    # (copy to SBUF first: approx recip does bitwise ops,
                    #  which are not valid on the PSUM fp32 read path)
                    den = npl.tile([1, 512], f32, tag="den", name=f"den{s}")
                    nc.vector.tensor_copy(den[:], p_o[64:65, :])
                    rec = npl.tile([1, 512], f32, tag="rec", name=f"rec{s}")
                    nc.vector.reciprocal_approx_fast(rec[:], den[:])
                    rec_b = npl.tile([64, 512], f32, tag="recb", name=f"recb{s}")
                    nc.gpsimd.partition_broadcast(rec_b[:], rec[:])
                    nc.vector.tensor_mul(
                        ot_t[m][po:po + 64, qs], p_o[0:64, :], rec_b[:])

            # ---- stages 0-5: lh1 projection groups ride under the first
            # exp calls ----
            fill1 = []
            for name, wt, tb, dst in (("k", twk, tbk, kt_t),
                                      ("q", twq, tbq, qt_t)):
                for m in range(2):
                    for lt in (2, 3):
                        fill1.append((name, wt, tb, dst, m, lt))
            fill_per_stage = [2, 2, 1, 1, 1, 1]
            s = 0
            for nfill in fill_per_stage:
                emit_scores(s)
                emit_exp(s)
                for _ in range(nfill):
                    args = fill1.pop(0)
                    proj_group(*args)
                s += 1
            pskq_cm.__exit__(None, None, None)

            # ---- stages 6-13: V projection rides under the exp stream ----
            nc.gpsimd.dma_start(twv[:], wv)
            x_chunks("v", vT, 0)
            x_chunks("v", vT, 1)
            nc.sync.dma_start(two[:], wo)

            ident = wp.tile([128, 128], bf16, tag="ident")
            masks.make_identity(nc, ident[:])
            vt_sb = [[qkp.tile([128, 512], bf16, tag=f"vtsb{m}_{lt}",
                               name=f"vtsb{m}_{lt}") for lt in range(LT)]
                     for m in range(2)]

            psv_cm = tc.tile_pool(name="ps_v", bufs=2, space="PSUM")
            psv = psv_cm.__enter__()

            def v_group(m, lt):
                pv = psv.tile([128, 512], f32, tag="pv", name=f"pv{m}_{lt}")
                lh, sl = lt // 2, (lt % 2)
                for e in range(ECH):
                    nc.tensor.matmul(
                        pv[:], twv[:, e, m * 128:(m + 1) * 128],
                        xch[("v", e % 2, e // 2, lh)][:, sl * 512:(sl + 1) * 512],
                        start=(e == 0), stop=(e == ECH - 1))
                nc.vector.tensor_copy(vt_sb[m][lt][:], pv[:])

            def transpose_batch(lt):
                # 8 PE transposes (4 kc x 2 m) into one borrowed psum bank,
                # then 2 wide DVE copies into the PV stationary layout.
                ptf = psv.tile([128, 512], f32, tag="pv", name=f"ptb{lt}")
                ptb = ptf[:].bitcast(bf16).rearrange("p (c m o) -> p c m o",
                                                     c=4, m=2)
                for c in range(4):
                    for m in range(2):
                        nc.tensor.transpose(
                            ptb[:, c, m, :], vt_sb[m][lt][:, c * 128:(c + 1) * 128],
                            ident[:])
                src5 = ptf[:].bitcast(bf16).rearrange(
                    "p (c m h d) -> p c m h d", c=4, m=2, h=2)
                for m in range(2):
                    nc.vector.tensor_copy(
                        v_t4[lt][:, :, 2 * m:2 * m + 2, 0:D], src5[:, :, m, :, :])

            vwork = [(m, lt) for lt in range(LT) for m in range(2)]
            for j, (m, lt) in enumerate(vwork):
                emit_scores(s)
                emit_exp(s)
                v_group(m, lt)
                if j % 2 == 1:
                    transpose_batch(lt)
                s += 1
            psv_cm.__exit__(None, None, None)

            # ================= steady attention ==================
            pso_cm = tc.tile_pool(name="ps_o", bufs=2, space="PSUM")
            pso = pso_cm.__enter__()
            pv_done = 0
            for s in range(s, NS):
                emit_scores(s)
                emit_exp(s)
                target = s - N_EARLY + 1
                if s >= NS - 3 * N_EARLY:
                    target = min(NS, s - N_EARLY + 2)
                if s >= NS - N_EARLY:
                    target = min(NS, s - N_EARLY + 3)
                while pv_done < min(target, s + 1):
                    emit_pv(pv_done, pso)
                    pv_done += 1
            while pv_done < NS:
                emit_pv(pv_done, pso)
                pv_done += 1
            pso_cm.__exit__(None, None, None)
            pst_cm.__exit__(None, None, None)

            # ================= output projection =================
            # per e-chunk: 4 psum tiles -> one wide SBUF tile -> one DMA.
            # oc-outer so each W_O stationary chunk is loaded once.
            with tc.tile_pool(name="ps_y", bufs=8, space="PSUM") as psy:
                cp_engs = [nc.vector, nc.scalar]
                for ec in range(ECH):
                    pys = [psy.tile([128, 512], f32, tag="y",
                                    name=f"py{ec}_{lt}") for lt in range(LT)]
                    tys = [ysp.tile([128, 1024], bf16, tag="ty",
                                    name=f"ty{ec}_{hl}") for hl in range(2)]
                    for lt in range(LT):
                        nc.tensor.matmul(
                            pys[lt][:], two[:, 0, ec * 128:(ec + 1) * 128],
                            ot_t[0][:, lt * 512:(lt + 1) * 512],
                            start=True, stop=False)
                    for lt in range(LT):
                        nc.tensor.matmul(
                            pys[lt][:], two[:, 1, ec * 128:(ec + 1) * 128],
                            ot_t[1][:, lt * 512:(lt + 1) * 512],
                            start=False, stop=True)
                        ty = tys[lt // 2]
                        eng = cp_engs[(ec + lt) % 2]
                        if eng is nc.scalar:
                            eng.copy(ty[:, (lt % 2) * 512:(lt % 2 + 1) * 512],
                                     pys[lt][:])
                        else:
                            eng.tensor_copy(
                                ty[:, (lt % 2) * 512:(lt % 2 + 1) * 512],
                                pys[lt][:])
                        if lt % 2 == 1:
                            hl = lt // 2
                            dma_out(yT[ec * 128:(ec + 1) * 128,
                                       hl * 1024:(hl + 1) * 1024], tys[hl][:])

    nc.compile()
    return nc


def _get_nc():
    if "nc" not in _CACHE:
        _CACHE["nc"] = _build()
    return _CACHE["nc"]


def _pack_x(xb):
    """[L, E] fp32 -> [128, 2, 4, L] bf16 with [p, h, c, l] = x[l, (c*2+h)*128+p]."""
    bf = ml_dtypes.bfloat16
    xT = xb.T.reshape(4, 2, 128, L)           # [c, h, p, l]
    return np.ascontiguousarray(xT.transpose(2, 1, 0, 3)).astype(bf)


def _make_in_maps(inputs):
    bf = ml_dtypes.bfloat16
    q = np.asarray(inputs["query"], dtype=np.float32)
    k = np.asarray(inputs["key"], dtype=np.float32)
    v = np.asarray(inputs["value"], dtype=np.float32)
    WQ = np.asarray(inputs["W_Query"], dtype=np.float32)
    WK = np.asarray(inputs["W_Key"], dtype=np.float32)
    WV = np.asarray(inputs["W_Value"], dtype=np.float32)
    WO = np.asarray(inputs["W_Output"], dtype=np.float32)
    BQ = np.asarray(inputs["B_Query"], dtype=np.float32)
    BK = np.asarray(inputs["B_Key"], dtype=np.float32)

    qp = [_pack_x(q[b]) for b in range(B)]
    kp = [_pack_x(k[b]) for b in range(B)]
    vp = [_pack_x(v[b]) for b in range(B)]

    in_maps = []
    for c in range(8):
        b, g = c // 4, c % 4
        sl = slice(OC * g, OC * (g + 1))
        # weights partition-major: [p, e, o] = W[e*128+p, o]
        wqp = np.ascontiguousarray(WQ[:, sl].reshape(ECH, 128, OC).transpose(1, 0, 2)).astype(bf)
        wkp = np.ascontiguousarray(WK[:, sl].reshape(ECH, 128, OC).transpose(1, 0, 2)).astype(bf)
        wvp = np.ascontiguousarray(WV[:, sl].reshape(ECH, 128, OC).transpose(1, 0, 2)).astype(bf)
        wop = np.ascontiguousarray(WO[sl, :].reshape(2, 128, E).transpose(1, 0, 2)).astype(bf)
        in_maps.append({
            "qT": qp[b],
            "kT": kp[b],
            "vT": vp[b],
            "wq": wqp,
            "wk": wkp,
            "wv": wvp,
            "wo": wop,
            "bq": np.ascontiguousarray(BQ[sl].reshape(2, 128, 1).transpose(1, 0, 2)),
            "bk": np.ascontiguousarray(BK[sl].reshape(2, 128, 1).transpose(1, 0, 2)),
        })
    return in_maps


def _combine(results, inputs):
    WO = np.asarray(inputs["W_Output"], dtype=np.float32)
    BV = np.asarray(inputs["B_Value"], dtype=np.float32)
    BO = np.asarray(inputs["B_Output"], dtype=np.float32)
    out = np.zeros((B, L, E), dtype=np.float32)
    for c in range(8):
        out[c // 4] += results[c]["yT"].astype(np.float32).T
    out += (BV @ WO + BO)[None, None, :]
    return out


def kernel(**inputs):
    from concourse.bass_utils import run_bass_kernel_spmd

    nc = _get_nc()
    in_maps = _make_in_maps(inputs)
    res = run_bass_kernel_spmd(nc, in_maps, list(range(8)))
    return _combine(res.results, inputs)
